# revision 11
# baseline (speedup 1.0000x reference)
"""GCN (4x GCNConv + global mean pool + MLP head) on 8 Trainium2 NeuronCores.

v4 — node sharding with range-batched dma_gather aggregation.

  - Host: relabel the 100k nodes into 8 cores x 98 windows x 128 slots via
    capacity-constrained bin packing (<= 4*128 in-edges per window). Edges
    are grouped per (address-range, window) — 5 fixed ranges keep dma_gather
    int16 indices in bounds — and each group is padded to whole 128-edge
    columns. Column layout is [range][window][col] so one dma_gather per
    (batch, range) fetches the edge rows of many windows at once: the GpSimd
    Q7 emits gather descriptors at ~8ns/row, which is THE bottleneck for
    message passing here, so batching minimizes per-call overhead.
      gidx [128, C*8] int16 — wrapped gather indices (k -> [k%16, k//16],
            replicated across the 8 Q7 core groups), range-relative
      S    [128, C*128] fp8(e4m3) — exact one-hot scatter matrices
            (S[p, c*128+d] = 1 iff edge slot (c,p) targets local dst d)
    The GCN normalization D^-1/2 (A+I) D^-1/2 is folded into per-node
    scales: sources are pre-scaled by dinv (host for x, M-step output cast
    on device for h), the per-dst dinv rides the post-aggregation
    activation's per-partition scale, and the self-loop is one identity
    matmul per window against dense local rows.
  - Layer 1 aggregates FIRST (out1 = (A~ x) @ W1): x is replicated input, so
    gathers read a host-staged x_full — no AllGather for the 512-wide h1.
    agg1 round-trips DRAM once for its transpose (xbar transposed read).
  - Layers 2..4: M-step (feat @ W via xbar-transposed reads), AllGather of
    the h' shard per slab (triggers fired in a burst after each M phase so
    they never head-of-line-block the gather queue), then the batched
    aggregation (S matmuls accumulating in PSUM).
  - Mean pool: PE matmuls P_w^T @ feat4_w, AllReduce, tiny MLP head.
"""

import heapq

import numpy as np
import ml_dtypes

# ---------------------------------------------------------------- constants
N_NODES = 100000
N_EDGES = 400000
N_GRAPHS = 64
DIMS = [(512, 512), (512, 256), (256, 128), (128, 64)]
N_CORES = 8
P = 128
W_WINDOWS = 98
SLOTS = W_WINDOWS * P          # 12544
SLOTS_ALL = SLOTS * N_CORES    # 100352
BF16 = ml_dtypes.bfloat16
FP8 = ml_dtypes.float8_e4m3
N_SLAB = 4       # AllGather slabs per layer
NR = 5           # gather address ranges (int16 index limit)
SUB = 4          # max edge columns per window before range split
NB1 = 5          # windows per batch, layer 1 (512-wide tiles)
NB2 = 14         # windows per batch, layers 2-4
LAG = 2          # batches M(l+1) trails A(l) by
LAG_H1 = 2       # batches h1 trails agg1 by
D4P = 128        # layer-4 padded width (gather rows must be %256B)
I16MAX = 32767


def _slabs(w_windows=W_WINDOWS):
    base = w_windows // N_SLAB
    rem = w_windows % N_SLAB
    out = []
    w0 = 0
    for s in range(N_SLAB):
        nwin = base + (1 if s < rem else 0)
        if nwin > 0:
            out.append((w0, nwin))
        w0 += nwin
    return out


def _ranges():
    base = SLOTS_ALL // NR
    rem = SLOTS_ALL % NR
    sizes = [base + (1 if r < rem else 0) for r in range(NR)]
    lo = np.concatenate([[0], np.cumsum(sizes)])
    assert max(sizes) <= I16MAX
    return lo


def _grid(nb, w_windows=W_WINDOWS):
    return [(w0, min(nb, w_windows - w0)) for w0 in range(0, w_windows, nb)]


# ---------------------------------------------------------------- host prep
def _pack_nodes(cost, sub_real):
    nb = N_CORES * W_WINDOWS
    cap = sub_real * P
    order = np.argsort(-cost, kind="stable")
    bin_load = np.zeros(nb, dtype=np.int64)
    bin_cnt = np.zeros(nb, dtype=np.int64)
    node_bin = np.full(len(cost), -1, dtype=np.int64)
    heap = [(0, b) for b in range(nb)]
    heapq.heapify(heap)
    stash = []
    for n in order:
        c = cost[n]
        stash.clear()
        placed = False
        while heap:
            load, b = heapq.heappop(heap)
            if bin_load[b] + c <= cap and bin_cnt[b] < P:
                bin_load[b] += c
                bin_cnt[b] += 1
                node_bin[n] = b
                if bin_cnt[b] < P:
                    heapq.heappush(heap, (bin_load[b], b))
                placed = True
                break
            elif bin_cnt[b] < P:
                stash.append((load, b))
        for it in stash:
            heapq.heappush(heap, it)
        if not placed:
            return None, None
    return node_bin, bin_load


def _preprocess(x, edge_index, batch):
    src = np.asarray(edge_index[0], dtype=np.int64)
    dst = np.asarray(edge_index[1], dtype=np.int64)
    batch = np.asarray(batch, dtype=np.int64)
    n = x.shape[0]

    indeg = np.bincount(dst, minlength=n).astype(np.int64)
    deg = indeg.astype(np.float64) + 1.0
    dinv = 1.0 / np.sqrt(deg)

    for sub in (SUB, SUB + 1, SUB + 2):
        node_bin, bin_load = _pack_nodes(indeg, sub)
        if node_bin is not None:
            break
    assert node_bin is not None

    nbins = N_CORES * W_WINDOWS
    order = np.argsort(-bin_load, kind="stable")
    bin_core = np.empty(nbins, dtype=np.int64)
    bin_win = np.empty(nbins, dtype=np.int64)
    for i, b in enumerate(order):
        rnd, k = divmod(i, N_CORES)
        c = k if rnd % 2 == 0 else N_CORES - 1 - k
        bin_core[b] = c
        bin_win[b] = rnd

    node_core = bin_core[node_bin]
    node_win = bin_win[node_bin]
    gkey = node_core * W_WINDOWS + node_win
    sort_idx = np.argsort(gkey, kind="stable")
    gsorted = gkey[sort_idx]
    grp_start = np.searchsorted(gsorted, np.arange(nbins))
    slot_in_win = np.empty(n, dtype=np.int64)
    slot_in_win[sort_idx] = np.arange(n) - grp_start[gsorted]
    assert slot_in_win.max() < P

    slabs = _slabs()
    win_slab = np.zeros(W_WINDOWS, dtype=np.int64)
    win_off = np.zeros(W_WINDOWS, dtype=np.int64)
    slab_off = np.zeros(N_SLAB, dtype=np.int64)
    slab_rows = np.zeros(N_SLAB, dtype=np.int64)
    off = 0
    for s, (w0, nwin) in enumerate(slabs):
        win_slab[w0:w0 + nwin] = s
        win_off[w0:w0 + nwin] = np.arange(nwin)
        slab_off[s] = off
        slab_rows[s] = nwin * P
        off += N_CORES * nwin * P
    node_grow = (slab_off[win_slab[node_win]]
                 + node_core * slab_rows[win_slab[node_win]]
                 + win_off[node_win] * P + slot_in_win)

    # ---- edge columns per (range, window)
    g_src = node_grow[src]
    e_core = node_core[dst]
    e_win = node_win[dst]
    e_dstp = slot_in_win[dst]
    r_lo = _ranges()
    e_range = np.searchsorted(r_lo[1:], g_src, side="right")
    cnt = np.zeros((N_CORES, NR, W_WINDOWS), dtype=np.int64)
    np.add.at(cnt, (e_core, e_range, e_win), 1)
    cols_rw = -(-cnt.max(axis=0) // P)          # [NR, W] shared plan
    colbase = np.zeros((NR, W_WINDOWS), dtype=np.int64)
    colbase.ravel()[1:] = np.cumsum(cols_rw.ravel())[:-1]
    total_cols = int(cols_rw.sum())

    key = (e_core * NR + e_range) * W_WINDOWS + e_win
    es = np.argsort(key, kind="stable")
    ks_ = key[es]
    gstart = np.searchsorted(ks_, np.arange(N_CORES * NR * W_WINDOWS))
    e_rank = np.empty(len(key), dtype=np.int64)
    e_rank[es] = np.arange(len(key)) - gstart[ks_]
    assert (e_rank < cols_rw[e_range, e_win] * P).all()
    e_col = colbase[e_range, e_win] + e_rank // P
    e_p = e_rank % P

    gidx_flat = np.zeros((N_CORES, total_cols * P), dtype=np.int64)
    gidx_flat[e_core, e_col * P + e_p] = g_src - r_lo[e_range]
    assert gidx_flat.max() <= I16MAX and gidx_flat.min() >= 0
    gidx_flat = gidx_flat.astype(np.int16)
    wrapped = gidx_flat.reshape(N_CORES, total_cols * P // 16, 16)
    gidx = np.ascontiguousarray(
        np.tile(wrapped.transpose(0, 2, 1), (1, 8, 1)))   # [8, 128, X]

    S = np.zeros((N_CORES, P, total_cols * P), dtype=FP8)
    S[e_core, e_p, e_col * P + e_dstp] = 1.0

    dinv_slot = np.ones((N_CORES, P, W_WINDOWS), dtype=np.float32)
    dinv_slot[node_core, slot_in_win, node_win] = dinv

    d0 = x.shape[1]
    xs = np.asarray(x, np.float64) * dinv[:, None]
    x_full = np.zeros((SLOTS_ALL, d0), dtype=BF16)
    x_full[node_grow] = xs.astype(BF16)
    x_self = np.zeros((N_CORES, SLOTS, d0), dtype=BF16)
    x_self.reshape(N_CORES * SLOTS, d0)[
        node_core * SLOTS + node_win * P + slot_in_win] = xs.astype(BF16)

    poolP = np.zeros((N_CORES, P, W_WINDOWS * N_GRAPHS), dtype=BF16)
    pc = node_win * N_GRAPHS + batch
    poolP[node_core, slot_in_win, pc] = 1.0

    cnts = np.bincount(batch, minlength=N_GRAPHS).astype(np.float32)
    inv_cnt = (1.0 / np.maximum(cnts, 1.0)).reshape(N_GRAPHS, 1)

    plan = dict(cols_rw=cols_rw, colbase=colbase, total_cols=total_cols,
                r_lo=r_lo)
    return dict(plan=plan, gidx=gidx, S=S, dinv_slot=dinv_slot,
                x_full=x_full, x_self=x_self, poolP=poolP, inv_cnt=inv_cnt)


def _plan_key(plan, has_bias):
    return (has_bias, tuple(plan["cols_rw"].ravel().tolist()))


# ---------------------------------------------------------------- device IR
def build_program(plan, has_bias, n_cores=N_CORES, w_windows=W_WINDOWS,
                  dims=DIMS, n_graphs=N_GRAPHS):
    from contextlib import ExitStack

    import concourse.bass as bass
    import concourse.tile as tile
    from concourse import bacc, mybir
    from concourse.masks import make_identity

    dt = mybir.dt
    f32, bf16, fp8 = dt.float32, dt.bfloat16, dt.float8e4
    i16 = dt.int16
    AF = mybir.ActivationFunctionType
    ALU = mybir.AluOpType
    W = w_windows
    slots = W * P
    slots_all = slots * n_cores
    G = n_graphs
    d_last = dims[-1][1]
    rg = [list(range(n_cores))]
    nlay = len(dims)
    d0 = dims[0][0]

    cols_rw = plan["cols_rw"]
    colbase = plan["colbase"]
    total_cols = plan["total_cols"]
    r_lo = plan["r_lo"]

    slabs = _slabs()
    goffs = []
    goff = 0
    for (w0s, nwin) in slabs:
        goffs.append(goff)
        goff += n_cores * nwin * P

    gw = {1: d0, 2: dims[1][1], 3: dims[2][1], 4: D4P}

    nc = bacc.Bacc("TRN2", target_bir_lowering=False, debug=False,
                   num_devices=n_cores)

    xfull_d = nc.dram_tensor("x_full", [slots_all, d0], bf16,
                             kind="ExternalInput")
    xself_d = nc.dram_tensor("x_self", [slots, d0], bf16,
                             kind="ExternalInput")
    gidx_d = nc.dram_tensor("gidx", [P, total_cols * 8], i16,
                            kind="ExternalInput")
    S_d = nc.dram_tensor("S", [P, total_cols * P], fp8, kind="ExternalInput")
    dinv_d = nc.dram_tensor("dinv", [P, W], f32, kind="ExternalInput")
    W_d = [nc.dram_tensor("W1", [d0, dims[0][1]], bf16, kind="ExternalInput"),
           nc.dram_tensor("W2", [dims[1][0], dims[1][1]], bf16,
                          kind="ExternalInput"),
           nc.dram_tensor("W3", [dims[2][0], dims[2][1]], bf16,
                          kind="ExternalInput"),
           nc.dram_tensor("W4", [dims[3][0], D4P], bf16,
                          kind="ExternalInput")]
    B_d = [nc.dram_tensor(f"B{i+1}", [P, do], f32, kind="ExternalInput")
           for i, (_, do) in enumerate(dims)]
    poolP_d = nc.dram_tensor("poolP", [P, W * G], bf16, kind="ExternalInput")
    Wl1_d = nc.dram_tensor("Wl1", [d_last, 32], f32, kind="ExternalInput")
    bl1_d = nc.dram_tensor("bl1", [32, 1], f32, kind="ExternalInput")
    Wl_d = nc.dram_tensor("Wl", [32, 2], f32, kind="ExternalInput")
    bl_d = nc.dram_tensor("bl", [2, 1], f32, kind="ExternalInput")
    invc_d = nc.dram_tensor("invc", [G, 1], f32, kind="ExternalInput")
    out_head = nc.dram_tensor("out_head", [2, G], f32, kind="ExternalOutput")

    agg1 = nc.dram_tensor("agg1", [slots, d0], bf16)
    feat = [None] + [nc.dram_tensor(f"feat{l}", [slots, dims[l - 1][1]], bf16)
                     for l in (1, 2, 3)]
    agin = {l: nc.dram_tensor(f"agin{l}", [slots, gw[l]], bf16)
            for l in (2, 3, 4)}
    agout = {l: nc.dram_tensor(f"agout{l}", [slots_all, gw[l]], bf16,
                               addr_space="Shared")
             for l in (2, 3, 4)}
    pool_in = nc.dram_tensor("pool_in", [G, d_last], f32)
    pool_out = nc.dram_tensor("pool_out", [G, d_last], f32,
                              addr_space="Shared")

    grids = {1: _grid(NB1), 2: _grid(NB2), 3: _grid(NB2), 4: _grid(NB2)}

    def batch_calls(w0, nbw):
        """(ncol, rbase, rspan, c0) per nonempty range for this batch."""
        out = []
        for r in range(NR):
            c0 = int(colbase[r, w0])
            ncol = int(cols_rw[r, w0:w0 + nbw].sum())
            if ncol > 0:
                out.append((ncol, int(r_lo[r]),
                            int(r_lo[r + 1] - r_lo[r]), c0, r))
        return out

    with tile.TileContext(nc) as tc, ExitStack() as ctx:
        const = ctx.enter_context(tc.tile_pool(name="const", bufs=1))
        g_pool = ctx.enter_context(tc.tile_pool(name="g", bufs=7))
        sl_pool = ctx.enter_context(tc.tile_pool(name="sl", bufs=2))
        xt_pool = ctx.enter_context(tc.tile_pool(name="xt", bufs=2))
        x1_pool = ctx.enter_context(tc.tile_pool(name="x1", bufs=3))
        h_pool = ctx.enter_context(tc.tile_pool(name="h", bufs=3))
        psum_a = ctx.enter_context(tc.tile_pool(name="pa", bufs=3,
                                                space="PSUM"))
        psum_m = ctx.enter_context(tc.tile_pool(name="pm", bufs=2,
                                                space="PSUM"))
        psum_s = ctx.enter_context(tc.tile_pool(name="ps", bufs=2,
                                                space="PSUM"))

        S_sb = const.tile([P, total_cols * P], fp8, name="S_sb")
        nc.sync.dma_start(S_sb[:], S_d.ap())
        gidx_sb = const.tile([P, total_cols * 8], i16, name="gidx_sb")
        nc.sync.dma_start(gidx_sb[:], gidx_d.ap())
        dinv_sb = const.tile([P, W], f32, name="dinv_sb")
        nc.sync.dma_start(dinv_sb[:], dinv_d.ap())

        iden_sb = const.tile([P, P], bf16, name="iden_sb")
        make_identity(nc, iden_sb[:])

        W_sb = []
        for l in range(nlay):
            di = dims[l][0]
            do = D4P if l == 3 else dims[l][1]
            ks = di // P
            t = const.tile([P, ks, do], bf16, name=f"W{l}_sb")
            nc.sync.dma_start(t[:], W_d[l].ap().rearrange(
                "(kt p) do -> p kt do", p=P))
            W_sb.append(t)
        B_sb = []
        for l, (_, do) in enumerate(dims):
            if has_bias[l]:
                t = const.tile([P, do], f32, name=f"B{l}_sb")
                nc.sync.dma_start(t[:], B_d[l].ap())
                B_sb.append(t)
            else:
                B_sb.append(None)

        feat4_sb = const.tile([P, W * d_last], bf16, name="feat4_sb")

        Wl1_sb = const.tile([d_last, 32], f32, name="Wl1_sb")
        nc.sync.dma_start(Wl1_sb[:], Wl1_d.ap())
        bl1_sb = const.tile([32, 1], f32, name="bl1_sb")
        nc.sync.dma_start(bl1_sb[:], bl1_d.ap())
        Wl_sb = const.tile([32, 2], f32, name="Wl_sb")
        nc.sync.dma_start(Wl_sb[:], Wl_d.ap())
        bl_sb = const.tile([2, 1], f32, name="bl_sb")
        nc.sync.dma_start(bl_sb[:], bl_d.ap())
        invc_sb = const.tile([G, 1], f32, name="invc_sb")
        nc.sync.dma_start(invc_sb[:], invc_d.ap())

        def emit_agg(l, b):
            """Aggregate batch b of layer l: psum = S^T g + self."""
            w0, nbw = grids[l][b]
            do_g = gw[l]
            do_l = dims[l - 1][1]
            src = xfull_d if l == 1 else agout[l]
            tiles = {}          # range -> (tile, c0)
            for (ncol, rbase, rspan, c0, r) in batch_calls(w0, nbw):
                gt = g_pool.tile([P, ncol, do_g], bf16, tag="g")
                nc.gpsimd.dma_gather(
                    out_ap=gt[:], in_ap=src.ap()[rbase:rbase + rspan, :],
                    idxs_ap=gidx_sb[:, c0 * 8:(c0 + ncol) * 8],
                    num_idxs=ncol * P, num_idxs_reg=ncol * P,
                    elem_size=do_g, single_packet=False)
                tiles[r] = (gt, c0)
            selfsrc = xself_d if l == 1 else agin[l]
            sl = sl_pool.tile([P, nbw, do_g], bf16, tag="sl")
            nc.sync.dma_start(
                sl[:], selfsrc.ap()[w0 * P:(w0 + nbw) * P, :].rearrange(
                    "(nb p) d -> p nb d", p=P))
            for wi in range(nbw):
                w = w0 + wi
                ps = psum_a.tile([P, do_l], f32, tag="pa")
                first = True
                for r in range(NR):
                    for j in range(int(cols_rw[r, w])):
                        gt, c0 = tiles[r]
                        scol = int(colbase[r, w]) + j
                        nc.tensor.matmul(
                            ps[:], lhsT=S_sb[:, scol * P:(scol + 1) * P],
                            rhs=gt[:, scol - c0, :do_l],
                            start=first, stop=False)
                        first = False
                nc.tensor.matmul(ps[:], lhsT=iden_sb[:],
                                 rhs=sl[:, wi, :do_l],
                                 start=first, stop=True)
                if l == 1:
                    a1 = h_pool.tile([P, d0], bf16, tag="a1")
                    nc.scalar.activation(a1[:], ps[:], AF.Copy,
                                         scale=dinv_sb[:, w:w + 1])
                    nc.scalar.dma_start(agg1.ap()[w * P:(w + 1) * P, :],
                                        a1[:])
                elif l < nlay:
                    ft = h_pool.tile([P, do_l], bf16, tag="ft")
                    if has_bias[l - 1]:
                        nc.vector.scalar_tensor_tensor(
                            out=ps[:], in0=ps[:],
                            scalar=dinv_sb[:, w:w + 1], in1=B_sb[l - 1][:],
                            op0=ALU.mult, op1=ALU.add)
                        nc.scalar.activation(ft[:], ps[:], AF.Relu)
                    else:
                        nc.scalar.activation(ft[:], ps[:], AF.Relu,
                                             scale=dinv_sb[:, w:w + 1])
                    nc.scalar.dma_start(feat[l].ap()[w * P:(w + 1) * P, :],
                                        ft[:])
                else:
                    if has_bias[l - 1]:
                        nc.vector.scalar_tensor_tensor(
                            out=ps[:], in0=ps[:],
                            scalar=dinv_sb[:, w:w + 1], in1=B_sb[l - 1][:],
                            op0=ALU.mult, op1=ALU.add)
                        nc.scalar.activation(
                            feat4_sb[:, w * d_last:(w + 1) * d_last], ps[:],
                            AF.Copy)
                    else:
                        nc.scalar.activation(
                            feat4_sb[:, w * d_last:(w + 1) * d_last], ps[:],
                            AF.Copy, scale=dinv_sb[:, w:w + 1])
            if l == 1:
                xt = x1_pool.tile([P, d0 // P, nbw * P], bf16, tag="xt1")
                nc.sync.dma_start_transpose(
                    xt[:], agg1.ap()[w0 * P:(w0 + nbw) * P, :])
                return xt
            return None

        def emit_h1(b, xt):
            """out1 = (A~x) @ W1 (+b1), ReLU -> feat1."""
            w0, nbw = grids[1][b]
            do = dims[0][1]
            ks = d0 // P
            for wi in range(nbw):
                w = w0 + wi
                psh = psum_m.tile([P, do], f32, tag="pm")
                for kt in range(ks):
                    nc.tensor.matmul(psh[:],
                                     lhsT=xt[:, kt, wi * P:(wi + 1) * P],
                                     rhs=W_sb[0][:, kt, :],
                                     start=(kt == 0), stop=(kt == ks - 1))
                if has_bias[0]:
                    nc.vector.tensor_tensor(out=psh[:], in0=psh[:],
                                            in1=B_sb[0][:], op=ALU.add)
                f1 = h_pool.tile([P, do], bf16, tag="f1")
                nc.scalar.activation(f1[:], psh[:], AF.Relu)
                nc.scalar.dma_start(feat[1].ap()[w * P:(w + 1) * P, :],
                                    f1[:])

        def emit_m(l, b):
            """M-step of layer l (2..4): agin_l = dinv * (feat_{l-1} @ W_l).
            Runs on the grid of the layer that produced feat_{l-1}."""
            w0, nbw = grids[l - 1][b]
            di = dims[l - 1][0]
            do = gw[l]
            ks = di // P
            xt = xt_pool.tile([P, ks, nbw * P], bf16, tag="xtm")
            nc.sync.dma_start_transpose(
                xt[:], feat[l - 1].ap()[w0 * P:(w0 + nbw) * P, :])
            hm = h_pool.tile([P, nbw, do], bf16, tag="hm")
            for wi in range(nbw):
                w = w0 + wi
                psm = psum_m.tile([P, do], f32, tag="pm")
                for kt in range(ks):
                    nc.tensor.matmul(psm[:],
                                     lhsT=xt[:, kt, wi * P:(wi + 1) * P],
                                     rhs=W_sb[l - 1][:, kt, :],
                                     start=(kt == 0), stop=(kt == ks - 1))
                nc.vector.tensor_scalar_mul(hm[:, wi, :], psm[:],
                                            dinv_sb[:, w:w + 1])
            nc.scalar.dma_start(
                agin[l].ap()[w0 * P:(w0 + nbw) * P, :].rearrange(
                    "(nb p) d -> p nb d", p=P), hm[:])

        def emit_ag(l):
            for si, (w0s, nwin) in enumerate(slabs):
                rows = nwin * P
                nc.gpsimd.collective_compute(
                    "AllGather", mybir.AluOpType.bypass, replica_groups=rg,
                    ins=[agin[l].ap()[w0s * P:w0s * P + rows, :]],
                    outs=[agout[l].ap()[goffs[si]:
                                        goffs[si] + n_cores * rows, :]])

        # ---- layer 1: aggregate-first; h1 and M(2) trail
        nb1 = len(grids[1])
        xts = {}
        for b in range(nb1):
            xts[b] = emit_agg(1, b)
            if b >= LAG_H1:
                emit_h1(b - LAG_H1, xts.pop(b - LAG_H1))
            if b >= LAG_H1 + LAG:
                emit_m(2, b - LAG_H1 - LAG)
        for b in range(nb1 - LAG_H1, nb1):
            emit_h1(b, xts.pop(b))
        for b in range(max(0, nb1 - LAG_H1 - LAG), nb1):
            emit_m(2, b)
        emit_ag(2)

        # ---- layers 2..4 (M(l+1) runs on grids[l])
        for l in (2, 3, 4):
            nbl = len(grids[l])
            for b in range(nbl):
                emit_agg(l, b)
                if l < nlay and b >= LAG:
                    emit_m(l + 1, b - LAG)
            if l < nlay:
                for b in range(max(0, nbl - LAG), nbl):
                    emit_m(l + 1, b)
                emit_ag(l + 1)

        # ---- mean pool (poolP loaded late into gather-pool slots)
        wh = W // 2
        poolA = g_pool.tile([P, wh * G], bf16, tag="g")
        nc.sync.dma_start(poolA[:], poolP_d.ap()[:, :wh * G])
        poolB = g_pool.tile([P, (W - wh) * G], bf16, tag="g")
        nc.sync.dma_start(poolB[:], poolP_d.ap()[:, wh * G:])
        pp = psum_s.tile([G, d_last], f32, name="pool_ps", tag="ps_small")
        for w in range(W):
            pl = poolA[:, w * G:(w + 1) * G] if w < wh else \
                poolB[:, (w - wh) * G:(w - wh + 1) * G]
            nc.tensor.matmul(pp[:], lhsT=pl,
                             rhs=feat4_sb[:, w * d_last:(w + 1) * d_last],
                             start=(w == 0), stop=(w == W - 1))
        pool_sb = const.tile([G, d_last], f32, name="pool_sb")
        nc.vector.tensor_copy(pool_sb[:], pp[:])
        nc.sync.dma_start(pool_in.ap(), pool_sb[:])
        nc.gpsimd.collective_compute(
            "AllReduce", mybir.AluOpType.add, replica_groups=rg,
            ins=[pool_in.ap()], outs=[pool_out.ap()])
        psum_sb = const.tile([G, d_last], f32, name="psum_sb")
        nc.sync.dma_start(psum_sb[:], pool_out.ap())
        pooled = const.tile([G, d_last], f32, name="pooled")
        nc.vector.tensor_scalar_mul(pooled[:], psum_sb[:], invc_sb[:, :1])

        ideng = const.tile([G, G], f32, name="ideng")
        make_identity(nc, ideng[:])
        pt_ps = psum_s.tile([d_last, G], f32, name="pt_ps", tag="ps_small")
        nc.tensor.transpose(pt_ps[:], pooled[:], ideng[:])
        pt = const.tile([d_last, G], f32, name="pt")
        nc.vector.tensor_copy(pt[:], pt_ps[:])
        ps1 = psum_s.tile([32, G], f32, name="ps1", tag="ps_small")
        nc.tensor.matmul(ps1[:], lhsT=Wl1_sb[:], rhs=pt[:])
        h1t = const.tile([32, G], f32, name="h1t")
        nc.scalar.activation(h1t[:], ps1[:], AF.Relu, bias=bl1_sb[:, :1])
        ps2 = psum_s.tile([2, G], f32, name="ps2", tag="ps_small")
        nc.tensor.matmul(ps2[:], lhsT=Wl_sb[:], rhs=h1t[:])
        oh = const.tile([2, G], f32, name="oh")
        nc.vector.tensor_scalar_add(oh[:], ps2[:], bl_sb[:, :1])
        nc.sync.dma_start(out_head.ap(), oh[:])

    nc.compile()
    return nc


# ---------------------------------------------------------------- entry
_CACHE = {}


def _make_in_maps(prep, inp):
    Ws = [np.asarray(inp[f"W{i+1}"]) for i in range(4)]
    bs = [np.asarray(inp[f"b{i+1}"]) for i in range(4)]
    W4p = np.zeros((DIMS[3][0], D4P), dtype=BF16)
    W4p[:, :DIMS[3][1]] = Ws[3].astype(BF16)
    in_maps = []
    for c in range(N_CORES):
        m = dict(
            x_full=prep["x_full"],
            x_self=prep["x_self"][c],
            gidx=prep["gidx"][c], S=prep["S"][c],
            dinv=np.ascontiguousarray(prep["dinv_slot"][c]),
            poolP=prep["poolP"][c], invc=prep["inv_cnt"],
            Wl1=np.asarray(inp["Wl1"], np.float32),
            bl1=np.asarray(inp["bl1"], np.float32).reshape(-1, 1),
            Wl=np.asarray(inp["Wl"], np.float32),
            bl=np.asarray(inp["bl"], np.float32).reshape(-1, 1),
        )
        for i, (wm, bv) in enumerate(zip(Ws, bs)):
            m[f"W{i+1}"] = W4p if i == 3 else wm.astype(BF16)
            m[f"B{i+1}"] = np.broadcast_to(
                np.asarray(bv, np.float32), (P, len(bv))).copy()
        in_maps.append(m)
    return in_maps


def kernel(x, edge_index, batch, W1, b1, W2, b2, W3, b3, W4, b4,
           Wl1, bl1, Wl, bl):
    from concourse import bass_utils

    x = np.asarray(x)
    prep = _preprocess(x, np.asarray(edge_index), np.asarray(batch))
    bs = [np.asarray(b) for b in (b1, b2, b3, b4)]
    has_bias = tuple(bool(np.any(b != 0)) for b in bs)

    key = _plan_key(prep["plan"], has_bias)
    if key not in _CACHE:
        _CACHE[key] = build_program(prep["plan"], has_bias)
    nc = _CACHE[key]

    inp = dict(W1=W1, b1=b1, W2=W2, b2=b2, W3=W3, b3=b3, W4=W4, b4=b4,
               Wl1=Wl1, bl1=bl1, Wl=Wl, bl=bl)
    in_maps = _make_in_maps(prep, inp)
    res = bass_utils.run_bass_kernel_spmd(
        nc, in_maps, core_ids=list(range(N_CORES)))
    out = res.results[0]["out_head"]
    return np.ascontiguousarray(out.T.astype(np.float32))
